# revision 1
# baseline (speedup 1.0000x reference)
"""Trainium2 Bass kernel for nn_CrossHeadProjection (sparse_attention).

ret[b,g,m,t,s] = sum_{m'} (I + A(t) + Bk(s))[m,m'] * x[b,g,m',t,s]
  A(t)  = qw2(t) @ qw1(t)^T + diag(qdd(t))          (t-dependent 8x8)
  Bk(s) = kw2(s) @ kw1(s)^T + diag(kdd(s))          (s-dependent 8x8)

Strategy (per core; 8 cores = 4 (b,g) pairs x 2 T-halves, no cross-core comm):
  Partition layout p = m*16 + t_sub (8 heads x 16 t's = 128 partitions),
  free dim = s.  The whole t-dependent side (incl. identity + qdd) is ONE
  block-diagonal PE matmul per slab.  The s-dependent side runs on DVE with
  partition-replicated weight rows; its 8-way head reduction + broadcast is
  a second PE matmul (block structure J = ones(8,8) (x) I_16), and the three
  rank-update tensors are accumulated into the PSUM result by identity
  matmuls.  ScalarE downloads PSUM -> SBUF.  DMA is batched in 8-slab chunks
  (2-4 MB per transfer).  All host-side packing (layout shuffles, weight
  prep, bf16 casts) happens in numpy before launch.
"""

import numpy as np
import ml_dtypes

import concourse.bass as bass
import concourse.mybir as mybir
import concourse.tile as tile
from concourse.bass_utils import run_bass_kernel_spmd
from concourse.tile import TileContext

BF16 = ml_dtypes.bfloat16

B, H, T, S = 2, 16, 1024, 1024
G, M, I = 2, 8, 2
TC = T // 2            # t-range per core
NSLAB = TC // 16       # 32 slabs of 16 t-positions
CHUNK = 8              # slabs per DMA batch
NCH = NSLAB // CHUNK
NCORES = 8
SC = 512               # s-chunk (one PSUM bank of f32)
NSC = S // SC

VARIANT = "acth0"      # compute-shape variant used by kernel()
OUT_BF16 = True        # device writes bf16; host upcasts to f32


def _legalize_waits(nc):
    """The walrus build in this env accepts at most ONE sync-wait per
    instruction; Tile attaches up to ~4.  Hoist extra waits onto same-engine
    NoOps placed immediately before the instruction (engines execute their
    stream in order, so this is semantically identical)."""
    ctr = 0
    for fn in nc.m.functions:
        for blk in fn.blocks:
            insts = list(blk.instructions)
            out: list = []
            changed = False
            for inst in insts:
                si = inst.sync_info
                waits = list(si.on_wait) if si is not None else []
                if len(waits) > 1:
                    changed = True
                    for w in waits[:-1]:
                        ctr += 1
                        out.append(
                            mybir.InstNoOp(
                                name=f"LEGW-{ctr}",
                                engine=inst.engine,
                                ins=[],
                                outs=[],
                                sync_info=mybir.SyncInfo(on_wait=[w], on_update=[]),
                            )
                        )
                    inst.sync_info = mybir.SyncInfo(
                        on_wait=[waits[-1]], on_update=list(si.on_update)
                    )
                out.append(inst)
            if changed:
                try:
                    blk.instructions = out
                except Exception:
                    blk.instructions.clear()
                    blk.instructions.extend(out)
    return nc


def _build(reps: int, hw_loop: bool = False, variant: str = VARIANT,
           out_bf16: bool = OUT_BF16):
    bf = mybir.dt.bfloat16
    f32 = mybir.dt.float32
    odt = bf if out_bf16 else f32
    nc = bass.Bass()

    xs_d = nc.dram_tensor("xs", [NCH, 128, CHUNK, S], bf, kind="ExternalInput")
    wa_d = nc.dram_tensor("wa", [128, NSLAB, 128], bf, kind="ExternalInput")
    wj_d = nc.dram_tensor("wj", [128, 128], bf, kind="ExternalInput")
    wi_d = nc.dram_tensor("wi", [128, 128], bf, kind="ExternalInput")
    k1b_d = nc.dram_tensor("k1b", [I, 128, S], bf, kind="ExternalInput")
    k2b_d = nc.dram_tensor("k2b", [I, 128, S], bf, kind="ExternalInput")
    kdb_d = nc.dram_tensor("kdb", [128, S], bf, kind="ExternalInput")
    out_d = nc.dram_tensor("out", [NCH, 128, CHUNK, S], odt, kind="ExternalOutput")

    with TileContext(nc) as tc:
        with (
            tc.tile_pool(name="wpool", bufs=1) as wpool,
            tc.tile_pool(name="xpool", bufs=2) as xpool,
            tc.tile_pool(name="ypool", bufs=3) as ypool,
            tc.tile_pool(name="zpool", bufs=3) as zpool,
            tc.tile_pool(name="opool", bufs=2) as opool,
            tc.tile_pool(name="rpool", bufs=2, space=bass.MemorySpace.PSUM) as rpool,
            tc.tile_pool(name="hpool", bufs=2, space=bass.MemorySpace.PSUM) as hpool,
        ):
            wa_t = wpool.tile([128, NSLAB, 128], bf)
            nc.sync.dma_start(out=wa_t[:], in_=wa_d[:])
            wj_t = wpool.tile([128, 128], bf)
            nc.sync.dma_start(out=wj_t[:], in_=wj_d[:])
            wi_t = wpool.tile([128, 128], bf)
            nc.sync.dma_start(out=wi_t[:], in_=wi_d[:])
            k1b_t = wpool.tile([128, I, S], bf)
            k2b_t = wpool.tile([128, I, S], bf)
            for i in range(I):
                nc.sync.dma_start(out=k1b_t[:, i, :], in_=k1b_d[i])
                nc.sync.dma_start(out=k2b_t[:, i, :], in_=k2b_d[i])
            kdb_t = wpool.tile([128, S], bf)
            nc.sync.dma_start(out=kdb_t[:], in_=kdb_d[:])

            def do_slab(slab, xs_j, ot_j):
                for sc in range(NSC):
                    sl = slice(sc * SC, (sc + 1) * SC)
                    xs_sl = xs_j[:, sl]
                    if variant == "dveonly":
                        for tag in ("y0", "y1", "z0", "z1", "zk"):
                            tt = ypool.tile([128, SC], bf, tag=tag)
                            nc.vector.tensor_mul(tt[:], xs_sl, k1b_t[:, 0, sl])
                        continue
                    if variant == "actonly":
                        for tag in ("h0c", "h1c", "dl"):
                            tt = ypool.tile([128, SC], bf, tag=tag)
                            nc.scalar.copy(out=tt[:], in_=xs_sl)
                        continue
                    if variant == "peonly":
                        h0 = hpool.tile([128, SC], f32)
                        nc.tensor.matmul(h0[:], wj_t[:], xs_sl, start=True, stop=True)
                        h1 = hpool.tile([128, SC], f32)
                        nc.tensor.matmul(h1[:], wj_t[:], xs_sl, start=True, stop=True)
                        rt = rpool.tile([128, SC], f32)
                        nc.tensor.matmul(
                            rt[:], wa_t[:, slab, :], xs_sl, start=True, stop=False
                        )
                        nc.tensor.matmul(rt[:], wi_t[:], xs_sl, start=False, stop=False)
                        nc.tensor.matmul(rt[:], wi_t[:], xs_sl, start=False, stop=False)
                        nc.tensor.matmul(rt[:], wi_t[:], xs_sl, start=False, stop=True)
                        continue
                    y0 = ypool.tile([128, SC], bf)
                    nc.vector.tensor_mul(y0[:], xs_sl, k1b_t[:, 0, sl])
                    y1 = ypool.tile([128, SC], bf)
                    nc.vector.tensor_mul(y1[:], xs_sl, k1b_t[:, 1, sl])
                    h0 = hpool.tile([128, SC], f32)
                    nc.tensor.matmul(h0[:], wj_t[:], y0[:], start=True, stop=True)
                    h1 = hpool.tile([128, SC], f32)
                    nc.tensor.matmul(h1[:], wj_t[:], y1[:], start=True, stop=True)
                    if variant in ("acth0", "acth01"):
                        h0sb = ypool.tile([128, SC], bf)
                        nc.scalar.copy(out=h0sb[:], in_=h0[:])
                        z0src = h0sb[:]
                    else:
                        z0src = h0[:]
                    if variant == "acth01":
                        h1sb = ypool.tile([128, SC], bf)
                        nc.scalar.copy(out=h1sb[:], in_=h1[:])
                        z1src = h1sb[:]
                    else:
                        z1src = h1[:]
                    z0 = zpool.tile([128, SC], bf)
                    nc.vector.tensor_mul(z0[:], z0src, k2b_t[:, 0, sl])
                    z1 = zpool.tile([128, SC], bf)
                    nc.vector.tensor_mul(z1[:], z1src, k2b_t[:, 1, sl])
                    zk = zpool.tile([128, SC], bf)
                    nc.vector.tensor_mul(zk[:], xs_sl, kdb_t[:, sl])
                    rt = rpool.tile([128, SC], f32)
                    nc.tensor.matmul(
                        rt[:], wa_t[:, slab, :], xs_sl, start=True, stop=False
                    )
                    nc.tensor.matmul(rt[:], wi_t[:], z0[:], start=False, stop=False)
                    nc.tensor.matmul(rt[:], wi_t[:], z1[:], start=False, stop=False)
                    nc.tensor.matmul(rt[:], wi_t[:], zk[:], start=False, stop=True)
                    nc.scalar.copy(out=ot_j[:, sl], in_=rt[:])

            def body(_i=None):
                if variant == "computeonly":
                    xt = xpool.tile([128, CHUNK, S], bf)
                    nc.sync.dma_start(out=xt[:], in_=xs_d[0])
                    ot = opool.tile([128, CHUNK, S], odt)
                    for c in range(NCH):
                        for j in range(CHUNK):
                            do_slab(c * CHUNK + j, xt[:, j, :], ot[:, j, :])
                    nc.sync.dma_start(out=out_d[0], in_=ot[:])
                    return
                for c in range(NCH):
                    xt = xpool.tile([128, CHUNK, S], bf)
                    nc.sync.dma_start(out=xt[:], in_=xs_d[c])
                    if variant == "dmapure":
                        nc.sync.dma_start(out=out_d[c], in_=xt[:])
                        continue
                    if variant == "dmain":
                        nc.sync.dma_start(out=out_d[c][:, :1, :64], in_=xt[:, :1, :64])
                        continue
                    ot = opool.tile([128, CHUNK, S], odt)
                    if variant == "dmaonly":
                        nc.scalar.copy(out=ot[:], in_=xt[:])
                    else:
                        for j in range(CHUNK):
                            do_slab(c * CHUNK + j, xt[:, j, :], ot[:, j, :])
                    nc.sync.dma_start(out=out_d[c], in_=ot[:])

            if hw_loop:
                with tc.For_i(
                    0,
                    reps,
                    1,
                    hint_engines=(mybir.EngineType.PE, mybir.EngineType.DVE),
                ) as i:
                    body(i)
            else:
                for _rep in range(reps):
                    body()
    return _legalize_waits(nc)


_CACHE: dict[tuple, bass.Bass] = {}


def _get_program(reps: int, hw_loop: bool = False, variant: str = VARIANT,
                 out_bf16: bool = OUT_BF16) -> bass.Bass:
    key = (reps, hw_loop, variant, out_bf16)
    if key not in _CACHE:
        _CACHE[key] = _build(reps, hw_loop, variant, out_bf16)
    return _CACHE[key]


def _pack_core(x, qw1, qw2, kw1, kw2, qdd, kdd, core):
    b, g, th = core >> 2, (core >> 1) & 1, core & 1
    t0 = th * TC
    xc = x.reshape(B, G, M, T, S)[b, g, :, t0 : t0 + TC, :]
    # [slab, p=(m,16), s] then chunked [NCH, 128, CHUNK, S]
    xs = xc.reshape(M, NSLAB, 16, S).transpose(1, 0, 2, 3).reshape(NSLAB, 128, S)
    xs = xs.reshape(NCH, CHUNK, 128, S).transpose(0, 2, 1, 3)

    q1 = qw1[b, t0 : t0 + TC, g]
    q2 = qw2[b, t0 : t0 + TC, g]
    qd = qdd[b, t0 : t0 + TC, g]
    Aq = np.einsum("tmi,tni->tmn", q2, q1)
    Aq[:, np.arange(M), np.arange(M)] += 1.0 + qd
    Aq5 = Aq.reshape(NSLAB, 16, M, M)
    W = np.einsum("stmn,tu->sntmu", Aq5, np.eye(16, dtype=np.float32))
    W = W.reshape(NSLAB, 128, 128).transpose(1, 0, 2)  # [p, slab, col]

    k1 = kw1[b, :, g]
    k2 = kw2[b, :, g]
    kd = kdd[b, :, g]
    k1b = np.repeat(k1.transpose(2, 1, 0), 16, axis=1)  # [I, 128, S]
    k2b = np.repeat(k2.transpose(2, 1, 0), 16, axis=1)
    kdb = np.repeat(kd.T, 16, axis=0)  # [128, S]

    wj = np.kron(np.ones((M, M), np.float32), np.eye(16, dtype=np.float32))
    wi = np.eye(128, dtype=np.float32)
    return {
        "xs": np.ascontiguousarray(xs).astype(BF16),
        "wa": np.ascontiguousarray(W).astype(BF16),
        "wj": wj.astype(BF16),
        "wi": wi.astype(BF16),
        "k1b": np.ascontiguousarray(k1b).astype(BF16),
        "k2b": np.ascontiguousarray(k2b).astype(BF16),
        "kdb": np.ascontiguousarray(kdb).astype(BF16),
    }


def _prepare_in_maps(inputs: dict) -> list:
    x = np.asarray(inputs["inputs"], np.float32)
    args = {
        k: np.asarray(v, np.float32) for k, v in inputs.items() if k != "inputs"
    }
    return [_pack_core(x, core=c, **args) for c in range(NCORES)]


def _execute(nc: bass.Bass, in_maps: list) -> np.ndarray:
    res = run_bass_kernel_spmd(nc, in_maps, list(range(NCORES)))
    out = np.empty((B, H, T, S), np.float32)
    ov = out.reshape(B, G, M, T, S)
    for c in range(NCORES):
        b, g, th = c >> 2, (c >> 1) & 1, c & 1
        t0 = th * TC
        od = np.asarray(res.results[c]["out"], np.float32)
        od = od.transpose(0, 2, 1, 3).reshape(NSLAB, 128, S)
        oc = od.reshape(NSLAB, M, 16, S).transpose(1, 0, 2, 3).reshape(M, TC, S)
        ov[b, g, :, t0 : t0 + TC, :] = oc
    return out


def _run(inputs: dict, reps: int = 1, hw_loop: bool = False,
         variant: str = VARIANT, out_bf16: bool = OUT_BF16) -> np.ndarray:
    return _execute(
        _get_program(reps, hw_loop, variant, out_bf16), _prepare_in_maps(inputs)
    )


def kernel(**inputs) -> np.ndarray:
    return _run(inputs, reps=1)

